# revision 11
# baseline (speedup 1.0000x reference)
"""Trainium2 Bass kernel for nn_AnchorFreeHead (ragged segment mean-pool +
residual MLP + L2-normalize + contrastive CE loss).

Sharding: data-parallel over the batch (video) dim B=8 — one batch per
NeuronCore. FeatureProj weights and text_feat are replicated. Each core
computes the partial loss sum over its P=128 segments; the 8 partial sums are
averaged on the host (equivalent to the all-reduce of the scalar loss).

Algorithm (per core, batch b):
  The reference's cumsum-then-gather segment mean-pool is reformulated as a
  dense masked matmul: seg_sum[p, d] = sum_t ind[t, p] * img[t, d], with the
  0/1 indicator ind[t, p] = (seg_start[p] <= t < seg_start[p]+seg_len[p])
  generated on-chip from an iota + two compare ops (all in fp32 on exact
  small integers — no data-dependent addressing anywhere). The text-embedding
  gather is likewise replaced by similarities against ALL C=200 text rows
  followed by one-hot masked reductions.
"""

import numpy as np
from contextlib import ExitStack

import concourse.bass as bass
import concourse.tile as tile
from concourse import bacc, masks, mybir
from concourse.bass_utils import run_bass_kernel_spmd

F32 = mybir.dt.float32
F32R = mybir.dt.float32r
BF16 = mybir.dt.bfloat16
I32 = mybir.dt.int32
OP = mybir.AluOpType
ACT = mybir.ActivationFunctionType

B, T, D, P, C, NEG, H = 8, 20000, 512, 128, 200, 3, 256
TT = 128          # rows per matmul tile (contraction chunk)
GT = 4            # tiles per DMA group (1 MiB per dma_start)


def build_kernel(t_len: int = T, img_bufs: int = 5):
    """Build the per-core Bass program. t_len is parameterized so the
    simulator tests can run a small version; the harness uses t_len=T."""
    nc = bacc.Bacc("TRN2", target_bir_lowering=False, debug=False, num_devices=8)

    img = nc.dram_tensor("img", [t_len, D], F32R, kind="ExternalInput")
    text = nc.dram_tensor("text", [C, D], F32, kind="ExternalInput")
    w1 = nc.dram_tensor("w1", [D, H], F32, kind="ExternalInput")
    w2 = nc.dram_tensor("w2", [H, D], F32, kind="ExternalInput")
    b1 = nc.dram_tensor("b1", [1, H], F32, kind="ExternalInput")
    b2 = nc.dram_tensor("b2", [1, D], F32, kind="ExternalInput")
    seg_start = nc.dram_tensor("seg_start", [1, P], I32, kind="ExternalInput")
    seg_len = nc.dram_tensor("seg_len", [1, P], I32, kind="ExternalInput")
    labels = nc.dram_tensor("labels", [1, P], I32, kind="ExternalInput")
    neg = nc.dram_tensor("neg_idx", [P, NEG], I32, kind="ExternalInput")
    out = nc.dram_tensor("out", [1, 1], F32, kind="ExternalOutput")

    n_groups, rem = divmod(t_len, TT * GT)
    assert rem % TT in (rem, 0) or True
    rem_tiles, rem_rows = divmod(rem, TT)
    total_mms = n_groups * GT + rem_tiles + (1 if rem_rows else 0)

    with tile.TileContext(nc) as tc, ExitStack() as ctx:
        con = ctx.enter_context(tc.tile_pool(name="con", bufs=1))
        ep = ctx.enter_context(tc.tile_pool(name="ep", bufs=1))
        img_pool = ctx.enter_context(tc.tile_pool(name="img", bufs=img_bufs))
        ind_pool = ctx.enter_context(tc.tile_pool(name="ind", bufs=3))
        ps_seg = ctx.enter_context(tc.tile_pool(name="ps_seg", bufs=1, space="PSUM"))
        ps_wk = ctx.enter_context(tc.tile_pool(name="ps_wk", bufs=2, space="PSUM"))
        ps_mlp = ctx.enter_context(tc.tile_pool(name="ps_mlp", bufs=1, space="PSUM"))

        # ---- one-time setup (critical path: the cf4/ndf4 chain) -------------
        ss_row = con.tile([1, P], I32)
        nc.scalar.dma_start(ss_row[:], seg_start[:])
        sl_row = con.tile([1, P], I32)
        nc.scalar.dma_start(sl_row[:], seg_len[:])
        ss_row_f = con.tile([1, P], F32)
        nc.vector.tensor_copy(ss_row_f[:], ss_row[:])
        sl_row_f = con.tile([1, P], F32)
        nc.vector.tensor_copy(sl_row_f[:], sl_row[:])
        ss_b = con.tile([128, P], F32)
        nc.gpsimd.partition_broadcast(ss_b[:], ss_row_f[:])
        sl_b = con.tile([128, P], F32)
        nc.gpsimd.partition_broadcast(sl_b[:], sl_row_f[:])

        # iota4[t, j, p] = t + TT*j   (value along partitions + j offset)
        iota4 = con.tile([128, GT, P], I32)
        nc.gpsimd.iota(iota4[:], pattern=[[TT, GT], [0, P]], base=0,
                       channel_multiplier=1)
        iota4_f = con.tile([128, GT, P], F32)
        nc.vector.tensor_copy(iota4_f[:], iota4[:])

        # Cf4[t, j, p]  = (t + TT*j) - seg_start[p]
        # nDf4[t, j, p] = Cf4 + 1 - seg_len[p]      (== -(len-1-(t-start)))
        cf4 = con.tile([128, GT, P], F32)
        ndf4 = con.tile([128, GT, P], F32)
        for j in range(GT):
            nc.vector.tensor_tensor(cf4[:, j, :], iota4_f[:, j, :], ss_b[:],
                                    op=OP.subtract)
            nc.vector.scalar_tensor_tensor(ndf4[:, j, :], cf4[:, j, :], 1.0,
                                           sl_b[:], op0=OP.add, op1=OP.subtract)

        # ---- main loop: seg_sum[p, d] via indicator matmuls -----------------
        psum_seg = ps_seg.tile([128, D], F32)
        mm = 0

        def do_group(g, full_tiles, rem_rows_):
            nonlocal mm
            t0 = g * TT * GT
            ntile = full_tiles + (1 if rem_rows_ else 0)
            grp = img_pool.tile([128, GT, D], F32R, tag="img")
            if full_tiles == GT:
                nc.sync.dma_start(
                    grp[:], img[t0:t0 + TT * GT, :].rearrange("(j p) d -> p j d", p=128))
            else:
                for j in range(full_tiles):
                    nc.sync.dma_start(grp[:, j, :], img[t0 + j * TT:t0 + (j + 1) * TT, :])
                if rem_rows_:
                    nc.sync.dma_start(grp[0:rem_rows_, full_tiles, :],
                                      img[t0 + full_tiles * TT:t0 + full_tiles * TT + rem_rows_, :])
            g1 = ind_pool.tile([128, GT, P], F32, tag="g1")
            ind = ind_pool.tile([128, GT, P], F32R, tag="ind")
            if full_tiles == GT:
                nc.vector.tensor_scalar(g1[:], cf4[:], -float(TT * GT * g), None,
                                        op0=OP.is_ge)
                nc.vector.scalar_tensor_tensor(ind[:], ndf4[:], -float(TT * GT * g),
                                               g1[:], op0=OP.is_le, op1=OP.mult)
            else:
                for j in range(ntile):
                    r = 128 if j < full_tiles else rem_rows_
                    nc.vector.tensor_scalar(g1[0:r, j, :], cf4[0:r, j, :],
                                            -float(TT * GT * g), None, op0=OP.is_ge)
                    nc.vector.scalar_tensor_tensor(ind[0:r, j, :], ndf4[0:r, j, :],
                                                   -float(TT * GT * g), g1[0:r, j, :],
                                                   op0=OP.is_le, op1=OP.mult)
            for j in range(ntile):
                r = 128 if j < full_tiles else rem_rows_
                nc.tensor.matmul(psum_seg[:], ind[0:r, j, :],
                                 grp[0:r, j, :],
                                 start=(mm == 0), stop=(mm == total_mms - 1))
                mm += 1

        for g in range(n_groups):
            do_group(g, GT, 0)
        if rem:
            do_group(n_groups, rem_tiles, rem_rows)

        # ---- deferred setup (only needed by the epilogue) -------------------
        identity = con.tile([128, 128], F32)
        masks.make_identity(nc, identity[:])

        # per-partition columns
        sl_col_i = con.tile([P, 1], I32)
        nc.scalar.dma_start(sl_col_i[:], seg_len.ap().rearrange("o p -> p o"))
        sl_col_f = con.tile([P, 1], F32)
        nc.vector.tensor_copy(sl_col_f[:], sl_col_i[:])
        recip_len = con.tile([P, 1], F32)
        nc.vector.reciprocal(recip_len[:], sl_col_f[:])

        lab_col_i = con.tile([P, 1], I32)
        nc.scalar.dma_start(lab_col_i[:], labels.ap().rearrange("o p -> p o"))
        lab_col = con.tile([P, 1], F32)
        nc.vector.tensor_copy(lab_col[:], lab_col_i[:])
        neg_col_i = con.tile([P, NEG], I32)
        nc.scalar.dma_start(neg_col_i[:], neg.ap())
        neg_col = con.tile([P, NEG], F32)
        nc.vector.tensor_copy(neg_col[:], neg_col_i[:])

        iota_c = con.tile([128, C], I32)
        nc.gpsimd.iota(iota_c[:], pattern=[[1, C]], base=0, channel_multiplier=0)
        iota_c_f = con.tile([128, C], F32)
        nc.vector.tensor_copy(iota_c_f[:], iota_c[:])

        # weights / biases / text
        w1_sb = con.tile([128, D // 128, H], F32)
        nc.scalar.dma_start(w1_sb[:], w1.ap().rearrange("(c k) h -> k c h", k=128))
        w2_sb = con.tile([128, H // 128, D], F32)
        nc.scalar.dma_start(w2_sb[:], w2.ap().rearrange("(c k) d -> k c d", k=128))
        b1_sb = con.tile([1, H], F32)
        nc.scalar.dma_start(b1_sb[:], b1.ap())
        b2_sb = con.tile([1, D], F32)
        nc.scalar.dma_start(b2_sb[:], b2.ap())
        ones_row = con.tile([1, 128], F32)
        nc.gpsimd.memset(ones_row[:], 1.0)
        ones_col = con.tile([128, 1], F32)
        nc.gpsimd.memset(ones_col[:], 1.0)

        txt0 = con.tile([128, D], F32)
        nc.scalar.dma_start(txt0[:], text[0:128, :])
        txt1 = con.tile([128, D], F32)
        nc.scalar.dma_start(txt1[0:C - 128, :], text[128:C, :])
        # textT[d, jd, c] = text[c, jd*128 + d]
        textT = con.tile([128, D // 128, C], F32)
        for jd in range(D // 128):
            pt = ps_wk.tile([128, 128], F32, tag="ps_wk")
            nc.tensor.transpose(pt[:, 0:128], txt0[:, jd * 128:(jd + 1) * 128],
                                identity[:])
            nc.vector.tensor_copy(textT[:, jd, 0:128], pt[:, 0:128])
            pt2 = ps_wk.tile([128, 128], F32, tag="ps_wk")
            nc.tensor.transpose(pt2[:, 0:C - 128],
                                txt1[0:C - 128, jd * 128:(jd + 1) * 128],
                                identity[0:C - 128, 0:C - 128])
            nc.vector.tensor_copy(textT[:, jd, 128:C], pt2[:, 0:C - 128])


        # ---- epilogue -------------------------------------------------------
        # vis = seg_sum / len
        vis = ep.tile([128, D], F32)
        nc.vector.tensor_scalar_mul(vis[:], psum_seg[:], recip_len[:])

        visT = ep.tile([128, D // 128, 128], F32)
        for jd in range(D // 128):
            pt = ps_wk.tile([128, 128], F32, tag="ps_wk")
            nc.tensor.transpose(pt[:], vis[:, jd * 128:(jd + 1) * 128], identity[:])
            nc.vector.tensor_copy(visT[:, jd, :], pt[:])

        h_ps = ps_mlp.tile([128, H], F32, tag="ps_mlp")
        for c in range(D // 128):
            nc.tensor.matmul(h_ps[:], visT[:, c, :],
                             w1_sb[:, c, :],
                             start=(c == 0), stop=False)
        nc.tensor.matmul(h_ps[:], ones_row[:], b1_sb[:],
                         start=False, stop=True)
        h_sb = ep.tile([128, H], F32)
        nc.vector.tensor_scalar_max(h_sb[:], h_ps[:], 0.0)

        hT = ep.tile([128, H // 128, 128], F32)
        for c in range(H // 128):
            pt = ps_wk.tile([128, 128], F32, tag="ps_wk")
            nc.tensor.transpose(pt[:], h_sb[:, c * 128:(c + 1) * 128], identity[:])
            nc.vector.tensor_copy(hT[:, c, :], pt[:])

        o_ps = ps_mlp.tile([128, D], F32, tag="ps_o")
        for c in range(H // 128):
            nc.tensor.matmul(o_ps[:], hT[:, c, :],
                             w2_sb[:, c, :],
                             start=(c == 0), stop=False)
        nc.tensor.matmul(o_ps[:], ones_row[:], b2_sb[:],
                         start=False, stop=True)

        ov = ep.tile([128, D], F32)
        nc.vector.tensor_tensor(ov[:], o_ps[:], vis[:], op=OP.add)

        # 1/||ov|| (the eps=1e-12 guard is vacuous at these magnitudes but free)
        sq = ep.tile([128, D], F32)
        ssq = ep.tile([128, 1], F32)
        nc.vector.scalar_tensor_tensor(sq[:], ov[:], 0.0, ov[:], op0=OP.add,
                                       op1=OP.mult, accum_out=ssq[:])
        nrm = ep.tile([128, 1], F32)
        nc.scalar.sqrt(nrm[:], ssq[:])
        nrm2 = ep.tile([128, 1], F32)
        nc.vector.tensor_scalar_max(nrm2[:], nrm[:], 1e-12)
        rnorm = ep.tile([128, 1], F32)
        nc.vector.reciprocal(rnorm[:], nrm2[:])

        ovT = ep.tile([128, D // 128, 128], F32)
        for jd in range(D // 128):
            pt = ps_wk.tile([128, 128], F32, tag="ps_wk")
            nc.tensor.transpose(pt[:], ov[:, jd * 128:(jd + 1) * 128], identity[:])
            nc.vector.tensor_copy(ovT[:, jd, :], pt[:])

        sim_ps = ps_mlp.tile([128, C], F32, tag="ps_sim")
        for c in range(D // 128):
            nc.tensor.matmul(sim_ps[:], ovT[:, c, :],
                             textT[:, c, :],
                             start=(c == 0), stop=(c == D // 128 - 1))
        sim = ep.tile([128, C], F32)
        nc.vector.tensor_scalar_mul(sim[:], sim_ps[:], rnorm[:])

        # logits[p, k] = sim[p, idx_k[p]] via one-hot masked reduction:
        # junk = (iota_c == idx_k) * sim, logits_k = sum(junk) along free
        logits = ep.tile([128, 1 + NEG], F32)
        for k in range(1 + NEG):
            idx_ap = lab_col[:] if k == 0 else neg_col[:, k - 1:k]
            junk = ep.tile([128, C], F32, tag="junk")
            nc.vector.scalar_tensor_tensor(
                junk[:], iota_c_f[:], idx_ap, sim[:], op0=OP.is_equal,
                op1=OP.mult, accum_out=logits[:, k:k + 1])

        # loss terms: logsumexp(logits) - logits[:, 0]
        negmx = ep.tile([128, 1], F32)
        nc.vector.tensor_reduce(negmx[:], logits[:], axis=mybir.AxisListType.X,
                                op=OP.max, negate=True)
        exps = ep.tile([128, 1 + NEG], F32)
        sumexp = ep.tile([128, 1], F32)
        nc.scalar.activation(exps[:], logits[:], ACT.Exp, bias=negmx[:], scale=1.0,
                             accum_out=sumexp[:])
        lse = ep.tile([128, 1], F32)
        nc.scalar.activation(lse[:], sumexp[:], ACT.Ln)
        t1 = ep.tile([128, 1], F32)
        nc.vector.tensor_tensor(t1[:], lse[:], negmx[:], op=OP.subtract)
        term = ep.tile([128, 1], F32)
        nc.vector.tensor_tensor(term[:], t1[:], logits[:, 0:1], op=OP.subtract)

        loss_ps = ps_wk.tile([1, 1], F32, tag="ps_loss")
        nc.tensor.matmul(loss_ps[:], term[:], ones_col[:], start=True, stop=True)
        loss_sb = ep.tile([1, 1], F32)
        nc.vector.tensor_copy(loss_sb[:], loss_ps[:])
        nc.sync.dma_start(out[:], loss_sb[:])

    nc.compile()
    return nc


def make_in_maps(image_feat, text_feat, W1, b1, W2, b2, seg_start, seg_len,
                 labels, neg_idx):
    f32 = np.float32
    i32 = np.int32
    return [
        {
            "img": np.ascontiguousarray(image_feat[c], dtype=f32),
            "text": np.ascontiguousarray(text_feat[c], dtype=f32),
            "w1": np.ascontiguousarray(W1, dtype=f32),
            "w2": np.ascontiguousarray(W2, dtype=f32),
            "b1": np.ascontiguousarray(b1, dtype=f32).reshape(1, H),
            "b2": np.ascontiguousarray(b2, dtype=f32).reshape(1, D),
            "seg_start": np.ascontiguousarray(seg_start[c], dtype=i32).reshape(1, P),
            "seg_len": np.ascontiguousarray(seg_len[c], dtype=i32).reshape(1, P),
            "labels": np.ascontiguousarray(labels[c], dtype=i32).reshape(1, P),
            "neg_idx": np.ascontiguousarray(neg_idx[c], dtype=i32).reshape(P, NEG),
        }
        for c in range(B)
    ]


_NC_CACHE = {}


def _get_nc():
    if "nc" not in _NC_CACHE:
        _NC_CACHE["nc"] = build_kernel(T)
    return _NC_CACHE["nc"]


def kernel(image_feat, text_feat, W1, b1, W2, b2, seg_start, seg_len, labels,
           neg_idx, _trace=False):
    nc = _get_nc()
    in_maps = make_in_maps(np.asarray(image_feat), np.asarray(text_feat),
                           np.asarray(W1), np.asarray(b1), np.asarray(W2),
                           np.asarray(b2), np.asarray(seg_start),
                           np.asarray(seg_len), np.asarray(labels),
                           np.asarray(neg_idx))
    res = run_bass_kernel_spmd(nc, in_maps, core_ids=list(range(B)), trace=_trace)
    total = sum(float(res.results[c]["out"][0, 0]) for c in range(B))
    loss = np.float32(total / (B * P))
    if _trace:
        return loss, res
    return loss
